# revision 9
# baseline (speedup 1.0000x reference)
"""Strided (residue-group) attention for Trainium2, SPMD across 8 NeuronCores.

Problem: x[B=2,S=4096,E=1024] -> qkv proj -> per-(batch,head,residue-group)
attention (stride 8 -> 8 groups of n=512 tokens) -> out proj.

Sharding: by (batch, residue-group).  B*stride = 16 group-instances; each of
the 8 cores owns 2 (batch,group) pairs = 1024 tokens and computes their FULL
output rows (it holds all 16 heads for its tokens).  The residue groups are
independent, so there are no cross-device collectives at all; the host
permutes tokens into group-major order on the way in and inverts on the way
out.

Device kernel design (per core):
  - Host pre-transposes x so the kernel receives xT [E, 1024tok] (contiguous
    DMA; tensor engine needs the contraction dim on partitions).
  - QKV: qT,kT produced feature-on-partition ([f,tok]); v produced
    token-on-partition ([tok,f]).  All matmuls in float32r (full-rate fp32).
  - scoresT[k,q] = kT.T-chunks @ qT per head; head pairs are row-packed on
    the PE array (K=64 each at array rows 0-63 / 64-127).
  - exp on ScalarE without max-subtraction (scores are O(+-8), exp is safe).
  - PV: lhsT = [v | ones] (even heads) or [ones | v] (odd heads) so one
    accumulation chain yields both o-rows and 64 replicated softmax
    denominator rows; GpSimd partition_broadcast moves the reciprocal row
    across the partition halves, DVE does recip + normalize.
  - out proj: lhsT = oT chunks, rhs = Wout rows -> natural [tok, E] output.
"""

import os

import numpy as np

B, S, E = 2, 4096, 1024
H, ST = 16, 8
DH = E // H  # 64
N = S // ST  # 512 tokens per residue group
NCORES = 8
GPC = (B * ST) // NCORES  # 2 (batch,group) pairs per core
TOK = GPC * N  # 1024 tokens per core
P = 128
EC = E // P  # 8 contraction chunks of 128
NB = N // P  # 4 token chunks of 128 per group
FB = 2  # feature blocks of 512 in E
SCALE = 1.0 / float(np.sqrt(DH))

_CACHE: dict = {}


def _build_nc():
    import concourse.bass as bass
    import concourse.bacc as bacc
    import concourse.tile as tile
    from concourse import mybir

    F32 = mybir.dt.float32
    F32R = mybir.dt.float32r
    ADD = mybir.AluOpType.add
    EXP = mybir.ActivationFunctionType.Exp

    nc = bacc.Bacc()
    xt = nc.declare_dram_parameter("xt", [E, TOK], F32R, isOutput=False)
    wq = nc.declare_dram_parameter("wq", [E, E], F32R, isOutput=False)
    wk = nc.declare_dram_parameter("wk", [E, E], F32R, isOutput=False)
    wv = nc.declare_dram_parameter("wv", [E, E], F32R, isOutput=False)
    wo = nc.declare_dram_parameter("wo", [E, E], F32R, isOutput=False)
    bq = nc.declare_dram_parameter("bq", [E], F32, isOutput=False)
    bk = nc.declare_dram_parameter("bk", [E], F32, isOutput=False)
    bv = nc.declare_dram_parameter("bv", [E], F32, isOutput=False)
    bo = nc.declare_dram_parameter("bo", [E], F32, isOutput=False)
    vones = nc.declare_dram_parameter("vones", [H * P], F32R, isOutput=False)
    out = nc.declare_dram_parameter("out", [TOK, E], F32, isOutput=True)

    with tile.TileContext(nc) as tc, (
        tc.tile_pool(name="const", bufs=1)
    ) as const, tc.tile_pool(name="xtp", bufs=1) as xtp, tc.tile_pool(
        name="wqkp", bufs=2
    ) as wqkp, tc.tile_pool(name="wvp", bufs=9) as wvp, tc.tile_pool(
        name="qtp", bufs=8
    ) as qtp, tc.tile_pool(name="ktp", bufs=8) as ktp, tc.tile_pool(
        name="vpp", bufs=4
    ) as vpp, tc.tile_pool(name="expp", bufs=2) as expp, tc.tile_pool(
        name="otp", bufs=8
    ) as otp, tc.tile_pool(name="recp", bufs=4) as recp, tc.tile_pool(
        name="outp", bufs=3
    ) as outp, tc.tile_pool(name="psmm", bufs=2, space="PSUM") as psmm, tc.tile_pool(
        name="pssc", bufs=2, space="PSUM"
    ) as pssc, tc.tile_pool(name="pso", bufs=2, space="PSUM") as psop:
        # ---- constants -------------------------------------------------
        bq_sb = const.tile([P, EC], F32)
        nc.sync.dma_start(out=bq_sb, in_=bq[:].rearrange("(c p) -> p c", p=P))
        bk_sb = const.tile([P, EC], F32)
        nc.sync.dma_start(out=bk_sb, in_=bk[:].rearrange("(c p) -> p c", p=P))
        # partition-broadcast copies of the v / out biases (feature on free)
        bv_bc = const.tile([P, E], F32)
        nc.gpsimd.dma_start(out=bv_bc, in_=bv[:].partition_broadcast(P))
        bo_bc = const.tile([P, E], F32)
        nc.gpsimd.dma_start(out=bo_bc, in_=bo[:].partition_broadcast(P))
        # Wout resident: [p, fb, dc, 512] = rows dc*128+p, cols fb*512+j
        wo_sb = const.tile([P, FB, EC, 512], F32R)
        for fb in range(FB):
            nc.sync.dma_start(
                out=wo_sb[:, fb],
                in_=wo[:, fb * 512 : (fb + 1) * 512].rearrange(
                    "(c p) f -> p c f", p=P
                ),
            )

        for g in range(GPC):
            # ---- load xT for this group's 512 tokens -------------------
            xt_g = xtp.tile([P, EC, N], F32R)
            nc.sync.dma_start(
                out=xt_g,
                in_=xt[:, g * N : (g + 1) * N].rearrange("(c p) n -> p c n", p=P),
            )

            # ---- q/k projections (feature-on-partition) ----------------
            qts: list = []
            kts: list = []
            for wmat, bias_sb, lst, tag in (
                (wq, bq_sb, qts, "qt"),
                (wk, bk_sb, kts, "kt"),
            ):
                for ft in range(EC):
                    wt = wqkp.tile([P, EC, P], F32R, tag="wqk")
                    nc.sync.dma_start(
                        out=wt,
                        in_=wmat[:, ft * P : (ft + 1) * P].rearrange(
                            "(c p) f -> p c f", p=P
                        ),
                    )
                    ps = psmm.tile([P, N], F32, tag="mm")
                    for c in range(EC):
                        nc.tensor.matmul(
                            ps,
                            lhsT=wt[:, c, :],
                            rhs=xt_g[:, c, :],
                            start=(c == 0),
                            stop=(c == EC - 1),
                        )
                    if tag == "qt":
                        t = qtp.tile([P, N], F32R, tag="qt")
                    else:
                        t = ktp.tile([P, N], F32R, tag="kt")
                    nc.vector.tensor_scalar(
                        out=t,
                        in0=ps,
                        scalar1=bias_sb[:, ft : ft + 1],
                        scalar2=None,
                        op0=ADD,
                    )
                    lst.append(t)

            # ---- v projection (token-on-partition, padded with ones) ---
            vts = []
            for tt in range(NB):
                vt = vpp.tile([P, H, P], F32R, tag="vp")
                # ones pattern (even heads: cols 64-127; odd: cols 0-63),
                # then the v-projection copies overwrite the v halves
                nc.gpsimd.dma_start(
                    out=vt,
                    in_=vones[:].rearrange("(h d) -> h d", h=H).partition_broadcast(P),
                )
                vts.append(vt)
            for fb in range(FB):
                wv_ts = []
                for c in range(EC):
                    wvt = wvp.tile([P, 512], F32R, tag="wv")
                    nc.sync.dma_start(
                        out=wvt, in_=wv[c * P : (c + 1) * P, fb * 512 : (fb + 1) * 512]
                    )
                    wv_ts.append(wvt)
                for tt in range(NB):
                    ps = psmm.tile([P, 512], F32, tag="mm")
                    for c in range(EC):
                        nc.tensor.matmul(
                            ps,
                            lhsT=xt_g[:, c, tt * P : (tt + 1) * P],
                            rhs=wv_ts[c],
                            start=(c == 0),
                            stop=(c == EC - 1),
                        )
                    for hl in range(8):
                        h = fb * 8 + hl
                        off = 0 if (h % 2 == 0) else DH
                        nc.vector.tensor_add(
                            out=vts[tt][:, h, off : off + DH],
                            in0=ps[:, hl * DH : (hl + 1) * DH],
                            in1=bv_bc[:, fb * 512 + hl * DH : fb * 512 + (hl + 1) * DH],
                        )

            # ---- attention per head pair -------------------------------
            ots = []
            for pr in range(EC):
                ex_A = expp.tile([P, NB, N], F32R, tag="exp")
                ex_B = expp.tile([P, NB, N], F32R, tag="exp")
                for half in range(2):
                    psA = pssc.tile([P, 2, N], F32, tag="sc")
                    psB = pssc.tile([P, 2, N], F32, tag="sc")
                    for cc in range(2):
                        c = 2 * half + cc
                        nc.tensor.matmul(
                            psA[:, cc],
                            lhsT=kts[pr][0:DH, c * P : (c + 1) * P],
                            rhs=qts[pr][0:DH, :],
                            start=True,
                            stop=True,
                        )
                        nc.tensor.matmul(
                            psB[:, cc],
                            lhsT=kts[pr][DH:P, c * P : (c + 1) * P],
                            rhs=qts[pr][DH:P, :],
                            start=True,
                            stop=True,
                        )
                    nc.scalar.activation(
                        out=ex_A[:, 2 * half : 2 * half + 2], in_=psA, func=EXP
                    )
                    nc.scalar.activation(
                        out=ex_B[:, 2 * half : 2 * half + 2], in_=psB, func=EXP
                    )
                ot = otp.tile([P, N], F32R, tag="ot")
                for h, ex in ((2 * pr, ex_A), (2 * pr + 1, ex_B)):
                    po = psop.tile([P, N], F32, tag="po")
                    for c in range(NB):
                        nc.tensor.matmul(
                            po,
                            lhsT=vts[c][:, h, :],
                            rhs=ex[:, c, :],
                            start=(c == 0),
                            stop=(c == NB - 1),
                        )
                    rec = recp.tile([P, N], F32, tag="rec")
                    rec2 = recp.tile([P, N], F32, tag="rec2")
                    if h % 2 == 0:
                        # o in rows 0-63, replicated denominator in 64-127;
                        # move the reciprocal row across the partition halves
                        # with a stride-0-replication DMA (engines cannot
                        # cross partitions)
                        nc.vector.reciprocal(out=rec[DH : DH + 1, :], in_=po[DH : DH + 1, :])
                        s = rec[DH : DH + 1, :]
                        nc.sync.dma_start(
                            out=rec2[0:DH, :],
                            in_=bass.AP(tensor=s.tensor, offset=s.offset,
                                        ap=[list(s.ap[0]), [0, DH], list(s.ap[1])]),
                        )
                        nc.vector.tensor_mul(
                            out=ot[0:DH, :], in0=po[0:DH, :], in1=rec2[0:DH, :]
                        )
                    else:
                        # denominator in rows 0-63, o in rows 64-127
                        nc.vector.reciprocal(out=rec[0:1, :], in_=po[0:1, :])
                        s = rec[0:1, :]
                        nc.sync.dma_start(
                            out=rec2[DH:P, :],
                            in_=bass.AP(tensor=s.tensor, offset=s.offset,
                                        ap=[list(s.ap[0]), [0, DH], list(s.ap[1])]),
                        )
                        nc.vector.tensor_mul(
                            out=ot[DH:P, :], in0=po[DH:P, :], in1=rec2[DH:P, :]
                        )
                ots.append(ot)

            # ---- out projection ---------------------------------------
            for tt in range(NB):
                for fb in range(FB):
                    ps = psmm.tile([P, 512], F32, tag="mm")
                    for dc in range(EC):
                        nc.tensor.matmul(
                            ps,
                            lhsT=ots[dc][:, tt * P : (tt + 1) * P],
                            rhs=wo_sb[:, fb, dc, :],
                            start=(dc == 0),
                            stop=(dc == EC - 1),
                        )
                    ob = outp.tile([P, 512], F32, tag="ob")
                    nc.vector.tensor_add(
                        out=ob, in0=ps, in1=bo_bc[:, fb * 512 : (fb + 1) * 512]
                    )
                    nc.sync.dma_start(
                        out=out[
                            g * N + tt * P : g * N + (tt + 1) * P,
                            fb * 512 : (fb + 1) * 512,
                        ],
                        in_=ob,
                    )
    nc.finalize()
    return nc


def _get_nc():
    if "nc" not in _CACHE:
        _CACHE["nc"] = _build_nc()
    return _CACHE["nc"]


def _make_in_maps(x, Wqkv, bqkv, Wout, bout):
    """Host-side sharding: permute tokens to group-major, pre-transpose x."""
    x = np.asarray(x, dtype=np.float32)
    Wqkv = np.asarray(Wqkv, dtype=np.float32)
    bqkv = np.asarray(bqkv, dtype=np.float32)
    Wout = np.ascontiguousarray(np.asarray(Wout, dtype=np.float32))
    bout = np.ascontiguousarray(np.asarray(bout, dtype=np.float32))

    # group-major token order: x_perm[b, g*N + i] = x[b, i*ST + g]
    x_perm = x.reshape(B, N, ST, E).transpose(0, 2, 1, 3)  # [B, ST, N, E]

    wq = np.ascontiguousarray(Wqkv[:, 0:E] * SCALE)
    wk = np.ascontiguousarray(Wqkv[:, E : 2 * E])
    wv = np.ascontiguousarray(Wqkv[:, 2 * E : 3 * E])
    bq = np.ascontiguousarray(bqkv[0:E] * SCALE)
    bk = np.ascontiguousarray(bqkv[E : 2 * E])
    bv = np.ascontiguousarray(bqkv[2 * E : 3 * E])

    vones = np.zeros(H * P, dtype=np.float32)
    for h in range(H):
        off = DH if h % 2 == 0 else 0
        vones[h * P + off : h * P + off + DH] = 1.0

    in_maps = []
    for c in range(NCORES):
        b = c // (NCORES // B)
        g0 = GPC * (c % (NCORES // B))
        xc = x_perm[b, g0 : g0 + GPC].reshape(TOK, E)  # [1024, E]
        xct = np.ascontiguousarray(xc.T)  # [E, 1024]
        in_maps.append(
            {
                "xt": xct,
                "wq": wq,
                "wk": wk,
                "wv": wv,
                "wo": Wout,
                "bq": bq,
                "bk": bk,
                "bv": bv,
                "bo": bout,
                "vones": vones,
            }
        )
    return in_maps


def kernel(x, Wqkv, bqkv, Wout, bout):
    from concourse.bass_utils import run_bass_kernel_spmd

    nc = _get_nc()
    in_maps = _make_in_maps(x, Wqkv, bqkv, Wout, bout)
    trace = bool(int(os.environ.get("KERNEL_TRACE", "0")))
    res = run_bass_kernel_spmd(
        nc, in_maps, core_ids=list(range(NCORES)), trace=trace
    )
    _CACHE["last_result"] = res

    # reassemble: core outputs are [1024 tok, E] in group-major token order
    out = np.empty((B, S, E), dtype=np.float32)
    for b in range(B):
        per_b = [res.results[b * (NCORES // B) + j]["out"] for j in range(NCORES // B)]
        perm = np.concatenate(per_b, axis=0)  # [ST*N, E] group-major
        out[b] = perm.reshape(ST, N, E).transpose(1, 0, 2).reshape(S, E)
    return out


# revision 11
# speedup vs baseline: 1.0278x; 1.0278x over previous
"""Strided (residue-group) attention for Trainium2, SPMD across 8 NeuronCores.

Problem: x[B=2,S=4096,E=1024] -> qkv proj -> per-(batch,head,residue-group)
attention (stride 8 -> 8 groups of n=512 tokens) -> out proj.

Sharding: by (batch, residue-group).  B*stride = 16 group-instances; each of
the 8 cores owns 2 (batch,group) pairs = 1024 tokens and computes their FULL
output rows (it holds all 16 heads for its tokens).  The residue groups are
independent, so there are no cross-device collectives at all; the host
permutes tokens into group-major order on the way in and inverts on the way
out.

Device kernel design (per core):
  - Host pre-transposes x so the kernel receives xT [E, 1024tok] (contiguous
    DMA; tensor engine needs the contraction dim on partitions).
  - QKV: qT,kT produced feature-on-partition ([f,tok]); v produced
    token-on-partition ([tok,f]).  All matmuls in float32r (full-rate fp32).
  - scoresT[k,q] = kT.T-chunks @ qT per head; head pairs are row-packed on
    the PE array (K=64 each at array rows 0-63 / 64-127).
  - exp on ScalarE without max-subtraction (scores are O(+-8), exp is safe).
  - PV: lhsT = [v | ones] (even heads) or [ones | v] (odd heads) so one
    accumulation chain yields both o-rows and 64 replicated softmax
    denominator rows; GpSimd partition_broadcast moves the reciprocal row
    across the partition halves, DVE does recip + normalize.
  - out proj: lhsT = oT chunks, rhs = Wout rows -> natural [tok, E] output.
"""

import os

import numpy as np

B, S, E = 2, 4096, 1024
H, ST = 16, 8
DH = E // H  # 64
N = S // ST  # 512 tokens per residue group
NCORES = 8
GPC = (B * ST) // NCORES  # 2 (batch,group) pairs per core
TOK = GPC * N  # 1024 tokens per core
P = 128
EC = E // P  # 8 contraction chunks of 128
NB = N // P  # 4 token chunks of 128 per group
FB = 2  # feature blocks of 512 in E
SCALE = 1.0 / float(np.sqrt(DH))

_CACHE: dict = {}


def _build_nc():
    import concourse.bass as bass
    import concourse.bacc as bacc
    import concourse.tile as tile
    from concourse import mybir

    F32 = mybir.dt.float32
    F32R = mybir.dt.float32r
    ADD = mybir.AluOpType.add
    EXP = mybir.ActivationFunctionType.Exp
    LOG = mybir.ActivationFunctionType.Ln

    nc = bacc.Bacc()
    xt = nc.declare_dram_parameter("xt", [E, TOK], F32R, isOutput=False)
    wq = nc.declare_dram_parameter("wq", [EC, P, EC, P], F32R, isOutput=False)
    wk = nc.declare_dram_parameter("wk", [EC, P, EC, P], F32R, isOutput=False)
    wv = nc.declare_dram_parameter("wv", [E, E], F32R, isOutput=False)
    wo = nc.declare_dram_parameter("wo", [E, E], F32R, isOutput=False)
    bq = nc.declare_dram_parameter("bq", [E], F32, isOutput=False)
    bk = nc.declare_dram_parameter("bk", [E], F32, isOutput=False)
    bv = nc.declare_dram_parameter("bv", [E], F32, isOutput=False)
    bo = nc.declare_dram_parameter("bo", [E], F32, isOutput=False)
    vones = nc.declare_dram_parameter("vones", [H * P], F32R, isOutput=False)
    out = nc.declare_dram_parameter("out", [TOK, E], F32, isOutput=True)

    with tile.TileContext(nc) as tc, (
        tc.tile_pool(name="const", bufs=1)
    ) as const, tc.tile_pool(name="xtp", bufs=1) as xtp, tc.tile_pool(
        name="wqkp", bufs=2
    ) as wqkp, tc.tile_pool(name="wvp", bufs=9) as wvp, tc.tile_pool(
        name="qtp", bufs=8
    ) as qtp, tc.tile_pool(name="ktp", bufs=8) as ktp, tc.tile_pool(
        name="vpp", bufs=4
    ) as vpp, tc.tile_pool(name="expp", bufs=2) as expp, tc.tile_pool(
        name="otp", bufs=8
    ) as otp, tc.tile_pool(name="recp", bufs=4) as recp, tc.tile_pool(
        name="outp", bufs=3
    ) as outp, tc.tile_pool(name="psmm", bufs=2, space="PSUM") as psmm, tc.tile_pool(
        name="pssc", bufs=2, space="PSUM"
    ) as pssc, tc.tile_pool(name="pso", bufs=2, space="PSUM") as psop:
        # ---- constants -------------------------------------------------
        bq_sb = const.tile([P, EC], F32)
        nc.sync.dma_start(out=bq_sb, in_=bq[:].rearrange("(c p) -> p c", p=P))
        bk_sb = const.tile([P, EC], F32)
        nc.sync.dma_start(out=bk_sb, in_=bk[:].rearrange("(c p) -> p c", p=P))
        # partition-broadcast copies of the v / out biases (feature on free)
        bv_bc = const.tile([P, E], F32)
        nc.gpsimd.dma_start(out=bv_bc, in_=bv[:].partition_broadcast(P))
        bo_bc = const.tile([P, E], F32)
        nc.gpsimd.dma_start(out=bo_bc, in_=bo[:].partition_broadcast(P))
        # Wout resident: [p, fb, dc, 512] = rows dc*128+p, cols fb*512+j
        wo_sb = const.tile([P, FB, EC, 512], F32R)
        for fb in range(FB):
            nc.sync.dma_start(
                out=wo_sb[:, fb],
                in_=wo[:, fb * 512 : (fb + 1) * 512].rearrange(
                    "(c p) f -> p c f", p=P
                ),
            )

        for g in range(GPC):
            # ---- load xT for this group's 512 tokens -------------------
            xt_g = xtp.tile([P, EC, N], F32R)
            nc.sync.dma_start(
                out=xt_g,
                in_=xt[:, g * N : (g + 1) * N].rearrange("(c p) n -> p c n", p=P),
            )

            # ---- q/k projections (feature-on-partition) ----------------
            qts: list = []
            kts: list = []
            for wmat, bias_sb, lst, tag in (
                (wq, bq_sb, qts, "qt"),
                (wk, bk_sb, kts, "kt"),
            ):
                for ft in range(EC):
                    wt = wqkp.tile([P, EC, P], F32R, tag="wqk")
                    nc.sync.dma_start(out=wt, in_=wmat[ft])
                    ps = psmm.tile([P, N], F32, tag="mm")
                    for c in range(EC):
                        nc.tensor.matmul(
                            ps,
                            lhsT=wt[:, c, :],
                            rhs=xt_g[:, c, :],
                            start=(c == 0),
                            stop=(c == EC - 1),
                        )
                    if tag == "qt":
                        t = qtp.tile([P, N], F32R, tag="qt")
                    else:
                        t = ktp.tile([P, N], F32R, tag="kt")
                    nc.vector.tensor_scalar(
                        out=t,
                        in0=ps,
                        scalar1=bias_sb[:, ft : ft + 1],
                        scalar2=None,
                        op0=ADD,
                    )
                    lst.append(t)

            # ---- v projection (token-on-partition, padded with ones) ---
            vts = []
            for tt in range(NB):
                vt = vpp.tile([P, H, P], F32R, tag="vp")
                # ones pattern (even heads: cols 64-127; odd: cols 0-63),
                # then the v-projection copies overwrite the v halves
                nc.gpsimd.dma_start(
                    out=vt,
                    in_=vones[:].rearrange("(h d) -> h d", h=H).partition_broadcast(P),
                )
                vts.append(vt)
            for fb in range(FB):
                wv_ts = []
                for c in range(EC):
                    wvt = wvp.tile([P, 512], F32R, tag="wv")
                    nc.sync.dma_start(
                        out=wvt, in_=wv[c * P : (c + 1) * P, fb * 512 : (fb + 1) * 512]
                    )
                    wv_ts.append(wvt)
                for tt in range(NB):
                    ps = psmm.tile([P, 512], F32, tag="mm")
                    for c in range(EC):
                        nc.tensor.matmul(
                            ps,
                            lhsT=xt_g[:, c, tt * P : (tt + 1) * P],
                            rhs=wv_ts[c],
                            start=(c == 0),
                            stop=(c == EC - 1),
                        )
                    for hl in range(8):
                        h = fb * 8 + hl
                        off = 0 if (h % 2 == 0) else DH
                        nc.vector.tensor_add(
                            out=vts[tt][:, h, off : off + DH],
                            in0=ps[:, hl * DH : (hl + 1) * DH],
                            in1=bv_bc[:, fb * 512 + hl * DH : fb * 512 + (hl + 1) * DH],
                        )

            # ---- attention per head pair -------------------------------
            ots = []
            for pr in range(EC):
                ex_A = expp.tile([P, NB, N], F32R, tag="exp")
                ex_B = expp.tile([P, NB, N], F32R, tag="exp")
                for half in range(2):
                    psA = pssc.tile([P, 2, N], F32, tag="sc")
                    psB = pssc.tile([P, 2, N], F32, tag="sc")
                    for cc in range(2):
                        c = 2 * half + cc
                        nc.tensor.matmul(
                            psA[:, cc],
                            lhsT=kts[pr][0:DH, c * P : (c + 1) * P],
                            rhs=qts[pr][0:DH, :],
                            start=True,
                            stop=True,
                        )
                        nc.tensor.matmul(
                            psB[:, cc],
                            lhsT=kts[pr][DH:P, c * P : (c + 1) * P],
                            rhs=qts[pr][DH:P, :],
                            start=True,
                            stop=True,
                        )
                    nc.scalar.activation(
                        out=ex_A[:, 2 * half : 2 * half + 2], in_=psA, func=EXP
                    )
                    nc.scalar.activation(
                        out=ex_B[:, 2 * half : 2 * half + 2], in_=psB, func=EXP
                    )
                ot = otp.tile([P, N], F32R, tag="ot")
                for h, ex in ((2 * pr, ex_A), (2 * pr + 1, ex_B)):
                    po = psop.tile([P, N], F32, tag="po")
                    for c in range(NB):
                        nc.tensor.matmul(
                            po,
                            lhsT=vts[c][:, h, :],
                            rhs=ex[:, c, :],
                            start=(c == 0),
                            stop=(c == NB - 1),
                        )
                    rec = recp.tile([P, N], F32, tag="rec")
                    rec2 = recp.tile([P, N], F32, tag="rec2")
                    if h % 2 == 0:
                        # o in rows 0-63, replicated denominator in 64-127;
                        # move the reciprocal row across the partition halves
                        # with a stride-0-replication DMA (engines cannot
                        # cross partitions)
                        nc.scalar.activation(out=rec[DH : DH + 1, :], in_=po[DH : DH + 1, :], func=LOG)
                        nc.scalar.activation(out=rec[DH : DH + 1, :], in_=rec[DH : DH + 1, :], func=EXP, scale=-1.0)
                        s = rec[DH : DH + 1, :]
                        nc.sync.dma_start(
                            out=rec2[0:DH, :],
                            in_=bass.AP(tensor=s.tensor, offset=s.offset,
                                        ap=[list(s.ap[0]), [0, DH], list(s.ap[1])]),
                        )
                        nc.vector.tensor_mul(
                            out=ot[0:DH, :], in0=po[0:DH, :], in1=rec2[0:DH, :]
                        )
                    else:
                        # denominator in rows 0-63, o in rows 64-127
                        nc.scalar.activation(out=rec[0:1, :], in_=po[0:1, :], func=LOG)
                        nc.scalar.activation(out=rec[0:1, :], in_=rec[0:1, :], func=EXP, scale=-1.0)
                        s = rec[0:1, :]
                        nc.sync.dma_start(
                            out=rec2[DH:P, :],
                            in_=bass.AP(tensor=s.tensor, offset=s.offset,
                                        ap=[list(s.ap[0]), [0, DH], list(s.ap[1])]),
                        )
                        nc.vector.tensor_mul(
                            out=ot[DH:P, :], in0=po[DH:P, :], in1=rec2[DH:P, :]
                        )
                ots.append(ot)

            # ---- out projection ---------------------------------------
            for tt in range(NB):
                for fb in range(FB):
                    ps = psmm.tile([P, 512], F32, tag="mm")
                    for dc in range(EC):
                        nc.tensor.matmul(
                            ps,
                            lhsT=ots[dc][:, tt * P : (tt + 1) * P],
                            rhs=wo_sb[:, fb, dc, :],
                            start=(dc == 0),
                            stop=(dc == EC - 1),
                        )
                    ob = outp.tile([P, 512], F32, tag="ob")
                    nc.vector.tensor_add(
                        out=ob, in0=ps, in1=bo_bc[:, fb * 512 : (fb + 1) * 512]
                    )
                    nc.sync.dma_start(
                        out=out[
                            g * N + tt * P : g * N + (tt + 1) * P,
                            fb * 512 : (fb + 1) * 512,
                        ],
                        in_=ob,
                    )
    nc.finalize()
    return nc


def _get_nc():
    if "nc" not in _CACHE:
        _CACHE["nc"] = _build_nc()
    return _CACHE["nc"]


def _make_in_maps(x, Wqkv, bqkv, Wout, bout):
    """Host-side sharding: permute tokens to group-major, pre-transpose x."""
    x = np.asarray(x, dtype=np.float32)
    Wqkv = np.asarray(Wqkv, dtype=np.float32)
    bqkv = np.asarray(bqkv, dtype=np.float32)
    Wout = np.ascontiguousarray(np.asarray(Wout, dtype=np.float32))
    bout = np.ascontiguousarray(np.asarray(bout, dtype=np.float32))

    # group-major token order: x_perm[b, g*N + i] = x[b, i*ST + g]
    x_perm = x.reshape(B, N, ST, E).transpose(0, 2, 1, 3)  # [B, ST, N, E]

    # [E, E] -> [ft, p, c, f] tile-major so each SBUF partition reads 4KB runs
    def tile_qk(w):
        return np.ascontiguousarray(
            w.reshape(EC, P, EC, P).transpose(2, 1, 0, 3)
        )

    wq = tile_qk(Wqkv[:, 0:E] * SCALE)
    wk = tile_qk(Wqkv[:, E : 2 * E])
    wv = np.ascontiguousarray(Wqkv[:, 2 * E : 3 * E])
    bq = np.ascontiguousarray(bqkv[0:E] * SCALE)
    bk = np.ascontiguousarray(bqkv[E : 2 * E])
    bv = np.ascontiguousarray(bqkv[2 * E : 3 * E])

    vones = np.zeros(H * P, dtype=np.float32)
    for h in range(H):
        off = DH if h % 2 == 0 else 0
        vones[h * P + off : h * P + off + DH] = 1.0

    in_maps = []
    for c in range(NCORES):
        b = c // (NCORES // B)
        g0 = GPC * (c % (NCORES // B))
        xc = x_perm[b, g0 : g0 + GPC].reshape(TOK, E)  # [1024, E]
        xct = np.ascontiguousarray(xc.T)  # [E, 1024]
        in_maps.append(
            {
                "xt": xct,
                "wq": wq,
                "wk": wk,
                "wv": wv,
                "wo": Wout,
                "bq": bq,
                "bk": bk,
                "bv": bv,
                "bo": bout,
                "vones": vones,
            }
        )
    return in_maps


def kernel(x, Wqkv, bqkv, Wout, bout):
    from concourse.bass_utils import run_bass_kernel_spmd

    nc = _get_nc()
    in_maps = _make_in_maps(x, Wqkv, bqkv, Wout, bout)
    trace = bool(int(os.environ.get("KERNEL_TRACE", "0")))
    res = run_bass_kernel_spmd(
        nc, in_maps, core_ids=list(range(NCORES)), trace=trace
    )
    _CACHE["last_result"] = res

    # reassemble: core outputs are [1024 tok, E] in group-major token order
    out = np.empty((B, S, E), dtype=np.float32)
    for b in range(B):
        per_b = [res.results[b * (NCORES // B) + j]["out"] for j in range(NCORES // B)]
        perm = np.concatenate(per_b, axis=0)  # [ST*N, E] group-major
        out[b] = perm.reshape(ST, N, E).transpose(1, 0, 2).reshape(S, E)
    return out
